# revision 44
# baseline (speedup 1.0000x reference)
"""Multi-head self-attention (B=4, L=2048, D=512, H=4, Hd=128) on 8 TRN2 cores.

Sharding: core c handles batch b = c//2 and head-pair p = c%2 (heads 2p, 2p+1).
Each core computes a partial output y_part[b] = sum_{h in pair} ctx_h @ Wo_h.T;
host gathers: y[b] = y_part[core 2b] + y_part[core 2b+1] + bo.

Dataflow per core (all matmuls bf16 inputs, fp32 PSUM accumulation; fp8 in
the value path was measured at 2e-2 rel err on this max-abs metric --
sharp-attention rows keep the full per-element quantization noise -- and
DoubleRow fp8 matmuls measured NO double-pump speedup on this hardware, so
everything stays bf16):
  xT [512,2048] (host-pretransposed)  ->  KT,QT [hd,L] and V [L,hd] via PE
      (K before Q: the first scores need all of KT but only one QT window;
       xT loads in [128,1024] pieces over all 3 DMA queues, weights first)
  scoresT [k,L_q] = KT_blk.T @ QT     (k-major: softmax along free dim never
  attnT = exp(scoresT/sqrt(hd))        needs a transpose anywhere)
  ctxT [hd,L_q] += V_blk.T @ attnT    (accumulate over k blocks)
  sm = at0+at1 (DVE), sm2 = sm0+sm1 (gpsimd): two fold levels quarter the
  r += ones.T @ sm2 rank-1 matmuls     (partition-dim reduce via matmul)
  rinv = recip_approx_fast(r)  (1 custom DVE op, ~5x faster than reciprocal)
  ctxT *= rinv  (rinv broadcast across partitions via DRAM-bounce DMA on the
                 SYNC queue -- gpsimd's in-order queue must stay pure compute
                 or its wait stalls the next iteration's folds; t=7 uses an
                 fp32 rank-1 PE broadcast to shorten the tail chain)
  y_blk [L_q,D] += ctxT_blk.T @ WoT_h (accumulate over the 2 heads; y out in
                 bf16 with the tail windows' DMAs split across queues)
Outproj is interleaved per query-window so PE never drains at the tail.
"""
import numpy as np
import ml_dtypes

B, L, D = 4, 2048, 512
H, HD = 4, 128
NCORES = 8
QW = 512          # query window (matmul N / PSUM bank)
NQC = L // QW     # 4 query windows
NKB = L // 128    # 16 key blocks
NDC = D // 128    # 4 contraction chunks for projections
SCALE = 1.0 / np.sqrt(HD)

_COMPILED = None


def _build():
    import concourse.bass as bass
    import concourse.mybir as mybir
    import concourse.tile as tile
    from concourse import bacc

    F32 = mybir.dt.float32
    F32R = mybir.dt.float32r
    BF16 = mybir.dt.bfloat16
    AF = mybir.ActivationFunctionType

    nc = bacc.Bacc("TRN2", target_bir_lowering=False, debug=False,
                   num_devices=NCORES)
    xT_d = nc.dram_tensor("xT", [D, L], BF16, kind="ExternalInput")
    wqT_d = nc.dram_tensor("wqT", [D, 256], BF16, kind="ExternalInput")
    wkT_d = nc.dram_tensor("wkT", [D, 256], BF16, kind="ExternalInput")
    wvT_d = nc.dram_tensor("wvT", [D, 256], BF16, kind="ExternalInput")
    woT_d = nc.dram_tensor("woT", [256, D], BF16, kind="ExternalInput")
    bq_d = nc.dram_tensor("bq", [128, 2], F32, kind="ExternalInput")
    bk_d = nc.dram_tensor("bk", [128, 2], F32, kind="ExternalInput")
    bv_d = nc.dram_tensor("bv", [1, 256], F32, kind="ExternalInput")
    y_d = nc.dram_tensor("y", [L, D], BF16, kind="ExternalOutput")

    with tile.TileContext(nc) as tc:
        with (
            tc.tile_pool(name="singles", bufs=1) as singles,
            tc.tile_pool(name="pss", bufs=2, space="PSUM") as pss_pool,
            tc.tile_pool(name="psc", bufs=1, space="PSUM") as psc_pool,
            tc.tile_pool(name="psr", bufs=1, space="PSUM") as psr_pool,
            tc.tile_pool(name="psy", bufs=2, space="PSUM") as psy_pool,
            tc.tile_pool(name="attnp", bufs=16) as attnp,
            tc.tile_pool(name="smp", bufs=20) as smp,
            tc.tile_pool(name="sm2p", bufs=10) as sm2p,
            tc.tile_pool(name="recp", bufs=4) as recp,
            tc.tile_pool(name="ctup", bufs=2) as ctup,
            tc.tile_pool(name="yp", bufs=3) as yp,
            tc.tile_pool(name="drp", bufs=4, space="DRAM") as drp,
        ):
            # ---- load inputs: xT is the 2MB critical path (per-queue DMA is
            # ~22GB/s) so it leads, split into [128,1024] pieces balanced over
            # all three DMA-capable queues; tiny wq/wk chunks go first so the
            # QK projection can consume xT pieces the moment they land ----
            qs = [nc.scalar, nc.gpsimd, nc.sync]
            wq_sb = singles.tile([128, NDC, 256], BF16)
            wk_sb = singles.tile([128, NDC, 256], BF16)
            wv_sb = singles.tile([128, NDC, 256], BF16)
            for c in range(NDC):
                qs[c % 2].dma_start(wq_sb[:, c, :],
                                    wqT_d[128 * c:128 * c + 128, :])
                qs[(c + 1) % 2].dma_start(wk_sb[:, c, :],
                                          wkT_d[128 * c:128 * c + 128, :])
            xt_sb = singles.tile([128, NDC, L], BF16)
            for i in range(2 * NDC):
                c, lh = i % NDC, i // NDC
                qs[i % 3].dma_start(
                    xt_sb[:, c, 1024 * lh:1024 * lh + 1024],
                    xT_d[128 * c:128 * c + 128, 1024 * lh:1024 * lh + 1024])
            for c in range(NDC):
                qs[(c + 2) % 3].dma_start(wv_sb[:, c, :],
                                          wvT_d[128 * c:128 * c + 128, :])
            bq_sb = singles.tile([128, 2], F32)
            bk_sb = singles.tile([128, 2], F32)
            nc.scalar.dma_start(bq_sb[:], bq_d[:])
            nc.gpsimd.dma_start(bk_sb[:], bk_d[:])
            bv_sb = singles.tile([128, 256], F32)
            nc.gpsimd.dma_start(
                bv_sb[:],
                bass.AP(tensor=bv_d.ap().tensor, offset=0, ap=[[0, 128], [1, 256]]))
            wo_sb = singles.tile([128, 2, D], BF16)
            for h in range(2):
                nc.sync.dma_start(wo_sb[:, h, :], woT_d[128 * h:128 * h + 128, :])
            ones_sb = singles.tile([128, 1], BF16)
            nc.vector.memset(ones_sb[:], 1.0)
            onesf_sb = singles.tile([1, 128], F32)
            nc.vector.memset(onesf_sb[:], 1.0)

            # PE warmup during the input-DMA window: dummy matmuls on memset
            # tiles lift HAM to 8/8 before the real burst arrives.
            warm_sb = singles.tile([128, 512], BF16)
            warmw_sb = singles.tile([128, 128], BF16)
            nc.vector.memset(warm_sb[:], 0.0)
            nc.vector.memset(warmw_sb[:], 0.0)
            for wi in range(16):
                ps_w = psy_pool.tile([128, D], F32, name=f"ps_w{wi}", tag="psy")
                nc.tensor.matmul(ps_w[:], warmw_sb[:], warm_sb[:],
                                 start=True, stop=True)

            # ---- Q/K projections ----
            qt_sb = singles.tile([128, 2, L], BF16)   # QT per head [hd, L]
            kt_sb = singles.tile([128, 2, L], BF16)
            v_sb = singles.tile([128, NKB, 256], BF16)  # V [k-part, kblk, 2*hd]

            # K before Q: t=0's scores need ALL of kt_h0 but only window 0
            # of qt_h0, so K-first unblocks the first scores sooner
            for h in range(2):
                for (w_sb, b_sb, o_sb) in ((wk_sb, bk_sb, kt_sb),
                                           (wq_sb, bq_sb, qt_sb)):
                    for qc in range(NQC):
                        win = slice(QW * qc, QW * qc + QW)
                        ps = pss_pool.tile([128, QW], F32,
                                           name=f"ps_p{h}{qc}", tag="pss")
                        for dc in range(NDC):
                            nc.tensor.matmul(
                                ps[:], w_sb[:, dc, 128 * h:128 * h + 128],
                                xt_sb[:, dc, win],
                                start=(dc == 0), stop=(dc == NDC - 1))
                        nc.vector.tensor_scalar_add(
                            o_sb[:, h, win], ps[:], b_sb[:, h:h + 1])

            def emit_vproj(lb):
                ps = pss_pool.tile([128, QW], F32, name=f"ps_v{lb}", tag="pss")
                for dc in range(NDC):
                    nc.tensor.matmul(
                        ps[:, 0:256], xt_sb[:, dc, 128 * lb:128 * lb + 128],
                        wv_sb[:, dc, :],
                        start=(dc == 0), stop=(dc == NDC - 1))
                nc.vector.tensor_add(v_sb[:, lb, :], ps[:, 0:256], bv_sb[:])

            # ---- attention: cross-iteration software pipeline ----
            # iteration t's PE stream: [ctx/rowsum of t-1 (dense, data ready)]
            # [finish t-1 off-PE] [outproj of an older window] [scores of t
            # (exp-paced stalls hidden behind the dense block)]
            ct_tiles = {}   # t -> [128, QW] bf16 normalized ctxT window

            def emit_scores_pair(t, qc, h, kk, at_tiles, sm2_tiles):
                win = slice(QW * qc, QW * qc + QW)
                ps_s = pss_pool.tile([128, 1024], F32,
                                     name=f"ps_s{t}_{kk}", tag="pss")
                k0 = 256 * kk
                nc.tensor.matmul(ps_s[:, 0:512], kt_sb[:, h, k0:k0 + 128],
                                 qt_sb[:, h, win], start=True, stop=True)
                nc.tensor.matmul(ps_s[:, 512:1024],
                                 kt_sb[:, h, k0 + 128:k0 + 256],
                                 qt_sb[:, h, win], start=True, stop=True)
                at = attnp.tile([128, 1024], BF16, name=f"at{t}_{kk}",
                                tag="attn")
                nc.scalar.activation(at[:], ps_s[:], AF.Exp, scale=SCALE)
                # fold the two k-chunks: rowsum is k-agnostic, one DVE add
                sm = smp.tile([128, 512], BF16, name=f"sm{t}_{kk}", tag="sm")
                nc.vector.tensor_add(sm[:], at[:, 0:512], at[:, 512:1024])
                at_tiles.append((at, sm))
                if kk % 2 == 1:
                    # second fold level on gpsimd (otherwise idle): halves
                    # the rank-1 rowsum matmul count again; emitted here so
                    # it has a full iteration of slack before the PE rank-1
                    # of the next iteration consumes it
                    sm2 = sm2p.tile([128, 512], BF16, name=f"sm2_{t}_{kk}",
                                    tag="sm2")
                    nc.gpsimd.tensor_add(sm2[:], at_tiles[kk - 1][1][:],
                                         sm[:])
                    sm2_tiles.append(sm2)

            def emit_ctx_pair(st, kk):
                t, h, at_tiles, sm2_tiles, ps_c, ps_r = st
                hs = slice(128 * h, 128 * h + 128)
                at, sm = at_tiles[kk]
                last = (kk == NKB // 2 - 1)
                nc.tensor.matmul(ps_c[:], v_sb[:, 2 * kk, hs], at[:, 0:512],
                                 start=(kk == 0), stop=False)
                nc.tensor.matmul(ps_c[:], v_sb[:, 2 * kk + 1, hs],
                                 at[:, 512:1024], start=False, stop=last)
                if kk % 2 == 1:
                    nc.tensor.matmul(ps_r[:], ones_sb[:],
                                     sm2_tiles[kk // 2][:],
                                     start=(kk == 1), stop=last)

            def finish_iter(st, fast):
                t, h, at_tiles, sm2_tiles, ps_c, ps_r = st
                # free the ctx PSUM bank immediately (no dependence on the
                # reciprocal chain) so the next iteration's ctx never waits
                ctu = ctup.tile([128, QW], F32, name=f"ctu{t}", tag="ctu")
                nc.vector.tensor_copy(ctu[:], ps_c[:])
                # reciprocal BEFORE the broadcast; approx_fast is one custom
                # DVE op at ~18 correct bits (vs ~3.4us for reciprocal())
                rinv = recp.tile([1, QW], F32, name=f"ri{t}", tag="ri")
                nc.vector.reciprocal_approx_fast(out=rinv[:], in_=ps_r[:])
                ct = singles.tile([128, QW], BF16, name=f"ct{t}")
                if fast:
                    # fp32 rank-1 PE broadcast: short chain for the tail
                    ps_b = psy_pool.tile([128, D], F32, name=f"ps_b{t}",
                                         tag="psy")
                    nc.tensor.matmul(ps_b[:, 0:QW], onesf_sb[:], rinv[:],
                                     start=True, stop=True)
                    nc.vector.tensor_mul(ct[:], ctu[:], ps_b[:, 0:QW])
                else:
                    # DRAM-bounce broadcast: zero PE cost, latency hidden by
                    # the one-iteration pipeline slack.  On the SYNC queue:
                    # gpsimd's in-order queue must stay pure compute or the
                    # bounce's wait blocks the next iteration's folds
                    rdram = drp.tile([1, QW], F32, name=f"rd{t}", tag="rd")
                    nc.sync.dma_start(rdram[:], rinv[:])
                    rd = rdram[:]
                    rb = bass.AP(tensor=rd.tensor, offset=rd.offset,
                                 ap=[[0, 128]] + [list(d) for d in rd.ap[1:]])
                    r128 = recp.tile([128, QW], F32, name=f"r128{t}",
                                     tag="r128")
                    nc.sync.dma_start(r128[:], rb)
                    nc.vector.tensor_mul(ct[:], ctu[:], r128[:])
                ct_tiles[t] = ct

            def emit_outproj1(qc, qq, split=False):
                th0, th1 = 2 * qc, 2 * qc + 1
                qb = NQC * qc + qq
                qsl = slice(128 * qq, 128 * qq + 128)
                ps_y = psy_pool.tile([128, D], F32, name=f"ps_y{qb}",
                                     tag="psy")
                nc.tensor.matmul(ps_y[:], ct_tiles[th0][:, qsl],
                                 wo_sb[:, 0, :], start=True, stop=False)
                nc.tensor.matmul(ps_y[:], ct_tiles[th1][:, qsl],
                                 wo_sb[:, 1, :], start=False, stop=True)
                ysb = yp.tile([128, D], BF16, name=f"ysb{qb}", tag="ysb")
                nc.vector.tensor_copy(ysb[:], ps_y[:])
                rows = slice(128 * qb, 128 * qb + 128)
                if split:
                    # tail windows: one queue moves 128KB at ~22GB/s (~6us),
                    # so split across queues to shorten the drain
                    nc.sync.dma_start(y_d[rows, 0:256], ysb[:, 0:256])
                    nc.gpsimd.dma_start(y_d[rows, 256:512], ysb[:, 256:512])
                else:
                    nc.sync.dma_start(y_d[rows, :], ysb[:])

            def emit_outproj2(qc, split=False):
                for qq in range(NQC):
                    emit_outproj1(qc, qq, split=split)

            prev = None
            for t in range(2 * NQC):
                qc, h = t // 2, t % 2
                ps_c = psc_pool.tile([128, QW], F32, name=f"ps_c{t}",
                                     tag="psc")
                ps_r = psr_pool.tile([1, QW], F32, name=f"ps_r{t}", tag="psr")
                at_tiles = []
                sm2_tiles = []
                fin = None
                for kk in range(NKB // 2):
                    if prev is not None:
                        emit_ctx_pair(prev, kk)
                        if kk == NKB // 2 - 1:
                            # finish prev right after its last ctx matmul so
                            # its DVE chain queues ahead of this t's last exp.
                            # Only t=7 (whose finish is on the tail critical
                            # path) uses the fp32 PE broadcast; t=6's bounce
                            # latency hides under the drain
                            fast = prev[0] >= 2 * NQC - 1
                            if not fast:
                                finish_iter(prev, fast=False)
                                fin = prev[0]
                    elif kk < NKB // 4:
                        # iteration 0: V projection fills the exp-paced slack
                        emit_vproj(2 * kk)
                        emit_vproj(2 * kk + 1)
                    emit_scores_pair(t, qc, h, kk, at_tiles, sm2_tiles)
                if prev is None:
                    for lb in range(NKB // 2, NKB):
                        emit_vproj(lb)
                else:
                    if fin is None:
                        finish_iter(prev, fast=True)
                    # two outproj windows per iteration from t=3 on levels
                    # PE load (4-window bursts at t=3/5 overfilled those
                    # iterations while later ones idled)
                    if t >= 3:
                        qb0 = 2 * (t - 3)
                        emit_outproj1(qb0 // 4, qb0 % 4)
                        emit_outproj1((qb0 + 1) // 4, (qb0 + 1) % 4)
                prev = (t, h, at_tiles, sm2_tiles, ps_c, ps_r)
            # drain the pipeline: outproj(2) leftovers fill exp-paced slips
            for kk in range(NKB // 2):
                emit_ctx_pair(prev, kk)
                if kk == 1:
                    emit_outproj1(2, 2, split=True)
                elif kk == 5:
                    emit_outproj1(2, 3, split=True)
            finish_iter(prev, fast=True)
            emit_outproj2(3, split=True)

    nc.compile()
    return nc


def _get_compiled():
    global _COMPILED
    if _COMPILED is None:
        _COMPILED = _build()
    return _COMPILED


def make_in_maps(x, Wq, bq, Wk, bk, Wv, bv, Wo):
    bf16 = ml_dtypes.bfloat16
    xT = {b: np.ascontiguousarray(x[b].T).astype(bf16) for b in range(B)}
    WqT, WkT, WvT, WoT = (np.ascontiguousarray(W.T) for W in (Wq, Wk, Wv, Wo))
    in_maps = []
    for c in range(NCORES):
        b = c // 2
        p = c % 2
        hs = slice(256 * p, 256 * p + 256)
        in_maps.append({
            "xT": xT[b],
            "wqT": WqT[:, hs].astype(bf16),
            "wkT": WkT[:, hs].astype(bf16),
            "wvT": WvT[:, hs].astype(bf16),
            "woT": np.ascontiguousarray(WoT[hs, :]).astype(bf16),
            "bq": np.ascontiguousarray(bq[hs].reshape(2, 128).T),
            "bk": np.ascontiguousarray(bk[hs].reshape(2, 128).T),
            "bv": bv[hs].reshape(1, 256).copy(),
        })
    return in_maps


def kernel(x, Wq, bq, Wk, bk, Wv, bv, Wo, bo):
    from concourse.bass_utils import run_bass_kernel_spmd

    x = np.asarray(x, np.float32)
    Wq, Wk, Wv, Wo = (np.asarray(w, np.float32) for w in (Wq, Wk, Wv, Wo))
    bq, bk, bv, bo = (np.asarray(b, np.float32) for b in (bq, bk, bv, bo))

    in_maps = make_in_maps(x, Wq, bq, Wk, bk, Wv, bv, Wo)
    nc = _get_compiled()
    try:
        res = run_bass_kernel_spmd(nc, in_maps, list(range(NCORES)))
    except Exception:
        # one retry: transient device wedges usually clear on re-execution
        res = run_bass_kernel_spmd(nc, in_maps, list(range(NCORES)))
    y = np.empty((B, L, D), np.float32)
    for b in range(B):
        y[b] = (res.results[2 * b]["y"].astype(np.float32)
                + res.results[2 * b + 1]["y"].astype(np.float32) + bo)
    return y


# revision 45
# speedup vs baseline: 1.1994x; 1.1994x over previous
"""Multi-head self-attention (B=4, L=2048, D=512, H=4, Hd=128) on 8 TRN2 cores.

Sharding: core c handles batch b = c//2 and head-pair p = c%2 (heads 2p, 2p+1).
Each core computes a partial output y_part[b] = sum_{h in pair} ctx_h @ Wo_h.T;
host gathers: y[b] = y_part[core 2b] + y_part[core 2b+1] + bo.

Dataflow per core (all matmuls bf16 inputs, fp32 PSUM accumulation; fp8 in
the value path was measured at 2e-2 rel err on this max-abs metric --
sharp-attention rows keep the full per-element quantization noise -- and
DoubleRow fp8 matmuls measured NO double-pump speedup on this hardware, so
everything stays bf16):
  xT [512,2048] (host-pretransposed)  ->  KT,QT [hd,L] and V [L,hd] via PE
      (K before Q: the first scores need all of KT but only one QT window;
       xT loads in [128,1024] pieces over all 3 DMA queues, weights first)
  scoresT [k,L_q] = KT_blk.T @ QT     (k-major: softmax along free dim never
  attnT = exp(scoresT/sqrt(hd))        needs a transpose anywhere)
  ctxT [hd,L_q] += V_blk.T @ attnT    (accumulate over k blocks)
  sm = at0+at1 (DVE), sm2 = sm0+sm1 (gpsimd): two fold levels quarter the
  r += ones.T @ sm2 rank-1 matmuls     (partition-dim reduce via matmul)
  rinv = recip_approx_fast(r)  (1 custom DVE op, ~5x faster than reciprocal)
  ctxT *= rinv  (rinv broadcast across partitions via DRAM-bounce DMA on the
                 SYNC queue -- gpsimd's in-order queue must stay pure compute
                 or its wait stalls the next iteration's folds; t=7 uses an
                 fp32 rank-1 PE broadcast to shorten the tail chain)
  y_blk [L_q,D] += ctxT_blk.T @ WoT_h (accumulate over the 2 heads; y out in
                 bf16 with the tail windows' DMAs split across queues)
Outproj is interleaved per query-window so PE never drains at the tail.
"""
import numpy as np
import ml_dtypes

B, L, D = 4, 2048, 512
H, HD = 4, 128
NCORES = 8
QW = 512          # query window (matmul N / PSUM bank)
NQC = L // QW     # 4 query windows
NKB = L // 128    # 16 key blocks
NDC = D // 128    # 4 contraction chunks for projections
SCALE = 1.0 / np.sqrt(HD)

_COMPILED = None


def _build():
    import concourse.bass as bass
    import concourse.mybir as mybir
    import concourse.tile as tile
    from concourse import bacc

    F32 = mybir.dt.float32
    F32R = mybir.dt.float32r
    BF16 = mybir.dt.bfloat16
    AF = mybir.ActivationFunctionType

    nc = bacc.Bacc("TRN2", target_bir_lowering=False, debug=False,
                   num_devices=NCORES)
    xT_d = nc.dram_tensor("xT", [D, L], BF16, kind="ExternalInput")
    wqT_d = nc.dram_tensor("wqT", [D, 256], BF16, kind="ExternalInput")
    wkT_d = nc.dram_tensor("wkT", [D, 256], BF16, kind="ExternalInput")
    wvT_d = nc.dram_tensor("wvT", [D, 256], BF16, kind="ExternalInput")
    woT_d = nc.dram_tensor("woT", [256, D], BF16, kind="ExternalInput")
    bq_d = nc.dram_tensor("bq", [128, 2], F32, kind="ExternalInput")
    bk_d = nc.dram_tensor("bk", [128, 2], F32, kind="ExternalInput")
    bv_d = nc.dram_tensor("bv", [1, 256], F32, kind="ExternalInput")
    y_d = nc.dram_tensor("y", [L, D], BF16, kind="ExternalOutput")

    with tile.TileContext(nc) as tc:
        with (
            tc.tile_pool(name="singles", bufs=1) as singles,
            tc.tile_pool(name="pss", bufs=2, space="PSUM") as pss_pool,
            tc.tile_pool(name="psc", bufs=1, space="PSUM") as psc_pool,
            tc.tile_pool(name="psr", bufs=1, space="PSUM") as psr_pool,
            tc.tile_pool(name="psy", bufs=2, space="PSUM") as psy_pool,
            tc.tile_pool(name="attnp", bufs=16) as attnp,
            tc.tile_pool(name="smp", bufs=20) as smp,
            tc.tile_pool(name="sm2p", bufs=10) as sm2p,
            tc.tile_pool(name="recp", bufs=4) as recp,
            tc.tile_pool(name="ctup", bufs=2) as ctup,
            tc.tile_pool(name="yp", bufs=3) as yp,
            tc.tile_pool(name="drp", bufs=4, space="DRAM") as drp,
        ):
            # ---- load inputs: xT is the 2MB critical path (per-queue DMA is
            # ~22GB/s) so it leads, split into [128,1024] pieces balanced over
            # all three DMA-capable queues; tiny wq/wk chunks go first so the
            # QK projection can consume xT pieces the moment they land ----
            qs = [nc.scalar, nc.gpsimd, nc.sync]
            wq_sb = singles.tile([128, NDC, 256], BF16)
            wk_sb = singles.tile([128, NDC, 256], BF16)
            wv_sb = singles.tile([128, NDC, 256], BF16)
            for c in range(NDC):
                qs[c % 2].dma_start(wq_sb[:, c, :],
                                    wqT_d[128 * c:128 * c + 128, :])
                qs[(c + 1) % 2].dma_start(wk_sb[:, c, :],
                                          wkT_d[128 * c:128 * c + 128, :])
            xt_sb = singles.tile([128, NDC, L], BF16)
            for i in range(2 * NDC):
                c, lh = i % NDC, i // NDC
                qs[i % 3].dma_start(
                    xt_sb[:, c, 1024 * lh:1024 * lh + 1024],
                    xT_d[128 * c:128 * c + 128, 1024 * lh:1024 * lh + 1024])
            for c in range(NDC):
                qs[(c + 2) % 3].dma_start(wv_sb[:, c, :],
                                          wvT_d[128 * c:128 * c + 128, :])
            bq_sb = singles.tile([128, 2], F32)
            bk_sb = singles.tile([128, 2], F32)
            nc.scalar.dma_start(bq_sb[:], bq_d[:])
            nc.gpsimd.dma_start(bk_sb[:], bk_d[:])
            bv_sb = singles.tile([128, 256], F32)
            nc.gpsimd.dma_start(
                bv_sb[:],
                bass.AP(tensor=bv_d.ap().tensor, offset=0, ap=[[0, 128], [1, 256]]))
            wo_sb = singles.tile([128, 2, D], BF16)
            for h in range(2):
                nc.sync.dma_start(wo_sb[:, h, :], woT_d[128 * h:128 * h + 128, :])
            ones_sb = singles.tile([128, 1], BF16)
            nc.vector.memset(ones_sb[:], 1.0)
            onesf_sb = singles.tile([1, 128], F32)
            nc.vector.memset(onesf_sb[:], 1.0)

            # PE warmup during the input-DMA window: dummy matmuls on memset
            # tiles lift HAM to 8/8 before the real burst arrives.
            warm_sb = singles.tile([128, 512], BF16)
            warmw_sb = singles.tile([128, 128], BF16)
            nc.vector.memset(warm_sb[:], 0.0)
            nc.vector.memset(warmw_sb[:], 0.0)
            for wi in range(16):
                ps_w = psy_pool.tile([128, D], F32, name=f"ps_w{wi}", tag="psy")
                nc.tensor.matmul(ps_w[:], warmw_sb[:], warm_sb[:],
                                 start=True, stop=True)

            # ---- Q/K projections ----
            qt_sb = singles.tile([128, 2, L], BF16)   # QT per head [hd, L]
            kt_sb = singles.tile([128, 2, L], BF16)
            v_sb = singles.tile([128, NKB, 256], BF16)  # V [k-part, kblk, 2*hd]

            # K before Q: t=0's scores need ALL of kt_h0 but only window 0
            # of qt_h0, so K-first unblocks the first scores sooner
            for h in range(2):
                for (w_sb, b_sb, o_sb) in ((wk_sb, bk_sb, kt_sb),
                                           (wq_sb, bq_sb, qt_sb)):
                    for qc in range(NQC):
                        win = slice(QW * qc, QW * qc + QW)
                        ps = pss_pool.tile([128, QW], F32,
                                           name=f"ps_p{h}{qc}", tag="pss")
                        for dc in range(NDC):
                            nc.tensor.matmul(
                                ps[:], w_sb[:, dc, 128 * h:128 * h + 128],
                                xt_sb[:, dc, win],
                                start=(dc == 0), stop=(dc == NDC - 1))
                        nc.vector.tensor_scalar_add(
                            o_sb[:, h, win], ps[:], b_sb[:, h:h + 1])

            def emit_vproj(lb):
                ps = pss_pool.tile([128, QW], F32, name=f"ps_v{lb}", tag="pss")
                for dc in range(NDC):
                    nc.tensor.matmul(
                        ps[:, 0:256], xt_sb[:, dc, 128 * lb:128 * lb + 128],
                        wv_sb[:, dc, :],
                        start=(dc == 0), stop=(dc == NDC - 1))
                nc.vector.tensor_add(v_sb[:, lb, :], ps[:, 0:256], bv_sb[:])

            # ---- attention: cross-iteration software pipeline ----
            # iteration t's PE stream: [ctx/rowsum of t-1 (dense, data ready)]
            # [finish t-1 off-PE] [outproj of an older window] [scores of t
            # (exp-paced stalls hidden behind the dense block)]
            ct_tiles = {}   # t -> [128, QW] bf16 normalized ctxT window

            def emit_scores_pair(t, qc, h, kk, at_tiles, sm2_tiles):
                win = slice(QW * qc, QW * qc + QW)
                ps_s = pss_pool.tile([128, 1024], F32,
                                     name=f"ps_s{t}_{kk}", tag="pss")
                k0 = 256 * kk
                nc.tensor.matmul(ps_s[:, 0:512], kt_sb[:, h, k0:k0 + 128],
                                 qt_sb[:, h, win], start=True, stop=True)
                nc.tensor.matmul(ps_s[:, 512:1024],
                                 kt_sb[:, h, k0 + 128:k0 + 256],
                                 qt_sb[:, h, win], start=True, stop=True)
                at = attnp.tile([128, 1024], BF16, name=f"at{t}_{kk}",
                                tag="attn")
                nc.scalar.activation(at[:], ps_s[:], AF.Exp, scale=SCALE)
                # fold the two k-chunks: rowsum is k-agnostic, one DVE add
                sm = smp.tile([128, 512], BF16, name=f"sm{t}_{kk}", tag="sm")
                nc.vector.tensor_add(sm[:], at[:, 0:512], at[:, 512:1024])
                at_tiles.append((at, sm))
                if kk % 2 == 1:
                    # second fold level on gpsimd (otherwise idle): halves
                    # the rank-1 rowsum matmul count again; emitted here so
                    # it has a full iteration of slack before the PE rank-1
                    # of the next iteration consumes it
                    sm2 = sm2p.tile([128, 512], BF16, name=f"sm2_{t}_{kk}",
                                    tag="sm2")
                    nc.gpsimd.tensor_add(sm2[:], at_tiles[kk - 1][1][:],
                                         sm[:])
                    sm2_tiles.append(sm2)

            def emit_ctx_pair(st, kk):
                t, h, at_tiles, sm2_tiles, ps_c, ps_r = st
                hs = slice(128 * h, 128 * h + 128)
                at, sm = at_tiles[kk]
                last = (kk == NKB // 2 - 1)
                nc.tensor.matmul(ps_c[:], v_sb[:, 2 * kk, hs], at[:, 0:512],
                                 start=(kk == 0), stop=False)
                nc.tensor.matmul(ps_c[:], v_sb[:, 2 * kk + 1, hs],
                                 at[:, 512:1024], start=False, stop=last)
                if kk % 2 == 1:
                    nc.tensor.matmul(ps_r[:], ones_sb[:],
                                     sm2_tiles[kk // 2][:],
                                     start=(kk == 1), stop=last)

            def finish_iter(st, fast):
                t, h, at_tiles, sm2_tiles, ps_c, ps_r = st
                # free the ctx PSUM bank immediately (no dependence on the
                # reciprocal chain) so the next iteration's ctx never waits
                ctu = ctup.tile([128, QW], F32, name=f"ctu{t}", tag="ctu")
                nc.vector.tensor_copy(ctu[:], ps_c[:])
                # reciprocal BEFORE the broadcast; approx_fast is one custom
                # DVE op at ~18 correct bits (vs ~3.4us for reciprocal())
                rinv = recp.tile([1, QW], F32, name=f"ri{t}", tag="ri")
                nc.vector.reciprocal_approx_fast(out=rinv[:], in_=ps_r[:])
                ct = singles.tile([128, QW], BF16, name=f"ct{t}")
                if fast:
                    # fp32 rank-1 PE broadcast: short chain for the tail
                    ps_b = psy_pool.tile([128, D], F32, name=f"ps_b{t}",
                                         tag="psy")
                    nc.tensor.matmul(ps_b[:, 0:QW], onesf_sb[:], rinv[:],
                                     start=True, stop=True)
                    nc.vector.tensor_mul(ct[:], ctu[:], ps_b[:, 0:QW])
                else:
                    # DRAM-bounce broadcast: zero PE cost, latency hidden by
                    # the one-iteration pipeline slack.  On the SYNC queue:
                    # gpsimd's in-order queue must stay pure compute or the
                    # bounce's wait blocks the next iteration's folds
                    rdram = drp.tile([1, QW], F32, name=f"rd{t}", tag="rd")
                    nc.sync.dma_start(rdram[:], rinv[:])
                    rd = rdram[:]
                    rb = bass.AP(tensor=rd.tensor, offset=rd.offset,
                                 ap=[[0, 128]] + [list(d) for d in rd.ap[1:]])
                    r128 = recp.tile([128, QW], F32, name=f"r128{t}",
                                     tag="r128")
                    nc.sync.dma_start(r128[:], rb)
                    nc.vector.tensor_mul(ct[:], ctu[:], r128[:])
                ct_tiles[t] = ct

            def emit_outproj1(qc, qq, split=False):
                th0, th1 = 2 * qc, 2 * qc + 1
                qb = NQC * qc + qq
                qsl = slice(128 * qq, 128 * qq + 128)
                ps_y = psy_pool.tile([128, D], F32, name=f"ps_y{qb}",
                                     tag="psy")
                nc.tensor.matmul(ps_y[:], ct_tiles[th0][:, qsl],
                                 wo_sb[:, 0, :], start=True, stop=False)
                nc.tensor.matmul(ps_y[:], ct_tiles[th1][:, qsl],
                                 wo_sb[:, 1, :], start=False, stop=True)
                ysb = yp.tile([128, D], BF16, name=f"ysb{qb}", tag="ysb")
                nc.vector.tensor_copy(ysb[:], ps_y[:])
                rows = slice(128 * qb, 128 * qb + 128)
                if split:
                    # tail windows: one queue moves 128KB at ~22GB/s (~6us),
                    # so split across queues to shorten the drain
                    nc.sync.dma_start(y_d[rows, 0:256], ysb[:, 0:256])
                    nc.gpsimd.dma_start(y_d[rows, 256:512], ysb[:, 256:512])
                else:
                    nc.sync.dma_start(y_d[rows, :], ysb[:])

            def emit_outproj2(qc, split=False):
                for qq in range(NQC):
                    emit_outproj1(qc, qq, split=split)

            prev = None
            for t in range(2 * NQC):
                qc, h = t // 2, t % 2
                ps_c = psc_pool.tile([128, QW], F32, name=f"ps_c{t}",
                                     tag="psc")
                ps_r = psr_pool.tile([1, QW], F32, name=f"ps_r{t}", tag="psr")
                at_tiles = []
                sm2_tiles = []
                fin = None
                for kk in range(NKB // 2):
                    if prev is not None:
                        emit_ctx_pair(prev, kk)
                        if kk == NKB // 2 - 1:
                            # finish prev right after its last ctx matmul so
                            # its DVE chain queues ahead of this t's last exp.
                            # Only t=7 (whose finish is on the tail critical
                            # path) uses the fp32 PE broadcast; t=6's bounce
                            # latency hides under the drain
                            fast = prev[0] >= 2 * NQC - 1
                            if not fast:
                                finish_iter(prev, fast=False)
                                fin = prev[0]
                    elif kk < NKB // 4:
                        # iteration 0: V projection fills the exp-paced slack
                        emit_vproj(2 * kk)
                        emit_vproj(2 * kk + 1)
                    emit_scores_pair(t, qc, h, kk, at_tiles, sm2_tiles)
                if prev is None:
                    for lb in range(NKB // 2, NKB):
                        emit_vproj(lb)
                else:
                    if fin is None:
                        finish_iter(prev, fast=True)
                    if t in (3, 5):
                        emit_outproj2((t - 3) // 2)
                prev = (t, h, at_tiles, sm2_tiles, ps_c, ps_r)
            # drain the pipeline: outproj(2) fills the exp-paced slips
            for kk in range(NKB // 2):
                emit_ctx_pair(prev, kk)
                if kk % 2 == 1:
                    emit_outproj1(2, kk // 2, split=True)
            finish_iter(prev, fast=True)
            emit_outproj2(3, split=True)

    nc.compile()
    return nc


def _get_compiled():
    global _COMPILED
    if _COMPILED is None:
        _COMPILED = _build()
    return _COMPILED


def make_in_maps(x, Wq, bq, Wk, bk, Wv, bv, Wo):
    bf16 = ml_dtypes.bfloat16
    xT = {b: np.ascontiguousarray(x[b].T).astype(bf16) for b in range(B)}
    WqT, WkT, WvT, WoT = (np.ascontiguousarray(W.T) for W in (Wq, Wk, Wv, Wo))
    in_maps = []
    for c in range(NCORES):
        b = c // 2
        p = c % 2
        hs = slice(256 * p, 256 * p + 256)
        in_maps.append({
            "xT": xT[b],
            "wqT": WqT[:, hs].astype(bf16),
            "wkT": WkT[:, hs].astype(bf16),
            "wvT": WvT[:, hs].astype(bf16),
            "woT": np.ascontiguousarray(WoT[hs, :]).astype(bf16),
            "bq": np.ascontiguousarray(bq[hs].reshape(2, 128).T),
            "bk": np.ascontiguousarray(bk[hs].reshape(2, 128).T),
            "bv": bv[hs].reshape(1, 256).copy(),
        })
    return in_maps


def kernel(x, Wq, bq, Wk, bk, Wv, bv, Wo, bo):
    from concourse.bass_utils import run_bass_kernel_spmd

    x = np.asarray(x, np.float32)
    Wq, Wk, Wv, Wo = (np.asarray(w, np.float32) for w in (Wq, Wk, Wv, Wo))
    bq, bk, bv, bo = (np.asarray(b, np.float32) for b in (bq, bk, bv, bo))

    in_maps = make_in_maps(x, Wq, bq, Wk, bk, Wv, bv, Wo)
    nc = _get_compiled()
    try:
        res = run_bass_kernel_spmd(nc, in_maps, list(range(NCORES)))
    except Exception:
        # one retry: transient device wedges usually clear on re-execution
        res = run_bass_kernel_spmd(nc, in_maps, list(range(NCORES)))
    y = np.empty((B, L, D), np.float32)
    for b in range(B):
        y[b] = (res.results[2 * b]["y"].astype(np.float32)
                + res.results[2 * b + 1]["y"].astype(np.float32) + bo)
    return y
